# revision 4
# baseline (speedup 1.0000x reference)
"""Trainium2 Bass kernel for nn_CoreDiffusion (gnn_message_passing).

Sharding: node dim N=4096 split across 8 cores (512 nodes each). Each core:
  msg[b,c] = adj[b,c,rows,:] @ x[b]   (fp16 operands, fp32 PSUM accum)
  hx[c] = relu(cumsum_c msg)          (fp32)
  GRU over c (float32r matmuls), sum over c, LayerNorm (fp32).
No collectives; full output gathered on host.
"""
import numpy as np
from contextlib import ExitStack

import concourse.bass as bass
import concourse.mybir as mybir
import concourse.tile as tile
from concourse import bacc
from concourse.masks import make_identity
from concourse.bass_utils import run_bass_kernel_spmd

F32 = mybir.dt.float32
F32R = mybir.dt.float32r
F16 = mybir.dt.float16
AF = mybir.ActivationFunctionType

B, C, N, D, H = 2, 4, 4096, 64, 64
NCORES = 8
NS = N // NCORES            # 512 nodes per core
JC = N // 128               # 32 contraction chunks
LN_EPS = 1e-5


def build():
    nc = bacc.Bacc("TRN2", target_bir_lowering=False, debug=False,
                   num_devices=NCORES)
    adj_s = nc.declare_dram_parameter("adj_s", [B, C, NS, N], F32, isOutput=False)
    x = nc.declare_dram_parameter("x", [B, N, D], F32, isOutput=False)
    w_ih = nc.declare_dram_parameter("w_ih", [3 * H, D], F32, isOutput=False)
    w_hh = nc.declare_dram_parameter("w_hh", [3 * H, H], F32, isOutput=False)
    b_ih = nc.declare_dram_parameter("b_ih", [3 * H], F32, isOutput=False)
    b_hh = nc.declare_dram_parameter("b_hh", [3 * H], F32, isOutput=False)
    gamma = nc.declare_dram_parameter("gamma", [H], F32, isOutput=False)
    beta = nc.declare_dram_parameter("beta", [H], F32, isOutput=False)
    out_s = nc.declare_dram_parameter("out_s", [B, NS, H], F32, isOutput=True)

    with tile.TileContext(nc) as tc, ExitStack() as ctx:
        const = ctx.enter_context(tc.tile_pool(name="const", bufs=1))
        adj_pool = ctx.enter_context(tc.tile_pool(name="adj", bufs=3))
        adjt_pool = ctx.enter_context(tc.tile_pool(name="adjt", bufs=4))
        gru = ctx.enter_context(tc.tile_pool(name="gru", bufs=2))
        psum = ctx.enter_context(tc.tile_pool(name="psum", bufs=1, space="PSUM"))
        psum_t = ctx.enter_context(tc.tile_pool(name="psum_t", bufs=2, space="PSUM"))
        psum_a = ctx.enter_context(tc.tile_pool(name="psum_a", bufs=2, space="PSUM"))

        # ---------- setup ----------
        ident = const.tile([128, 128], F32)
        make_identity(nc, ident)
        ident16 = const.tile([128, 128], F16)
        nc.vector.tensor_copy(ident16, ident)

        # x -> fp16, layout [j%128, jc, b, d]
        x16 = const.tile([128, JC, B, D], F16)
        for b in range(B):
            nc.gpsimd.dma_start(
                out=x16[:, :, b, :],
                in_=x[b].rearrange("(c p) d -> p c d", p=128))

        # GRU weights: load [192,64] as two partition blocks, PE-transpose gates
        wih_sb = const.tile([128, 2, D], F32)
        nc.sync.dma_start(wih_sb[:, 0, :], w_ih[0:128, :])
        nc.sync.dma_start(wih_sb[0:64, 1, :], w_ih[128:192, :])
        whh_sb = const.tile([128, 2, H], F32)
        nc.sync.dma_start(whh_sb[:, 0, :], w_hh[0:128, :])
        nc.sync.dma_start(whh_sb[0:64, 1, :], w_hh[128:192, :])
        # wT[:, 0:3] = w_ih^T gates r,z,n ; wT[:, 3:6] = w_hh^T
        wT = const.tile([64, 6, 64], F32R)
        for gi, (src, blk, prow) in enumerate([
                (wih_sb, 0, 0), (wih_sb, 0, 64), (wih_sb, 1, 0),
                (whh_sb, 0, 0), (whh_sb, 0, 64), (whh_sb, 1, 0)]):
            ps_w = psum_a.tile([64, 64], F32, tag="acc")
            nc.tensor.transpose(ps_w, src[prow:prow + 64, blk, :],
                                ident[prow:prow + 64, prow:prow + 64])
            nc.vector.tensor_copy(wT[:, gi, :], ps_w)

        # biases as [64, 3] (partition = gate-internal dim)
        bsum = const.tile([64, 3], F32)
        bih_sb = const.tile([64, 3], F32)
        nc.sync.dma_start(bih_sb, b_ih.rearrange("(g p) -> p g", p=64))
        bhh_sb = const.tile([64, 3], F32)
        nc.sync.dma_start(bhh_sb, b_hh.rearrange("(g p) -> p g", p=64))
        nc.vector.tensor_add(bsum, bih_sb, bhh_sb)

        gam_sb = const.tile([128, H], F32)
        g_ap = gamma[:]
        nc.gpsimd.dma_start(out=gam_sb, in_=bass.AP(
            tensor=g_ap.tensor, offset=g_ap.offset, ap=[[0, 128]] + list(g_ap.ap)))
        bet_sb = const.tile([128, H], F32)
        b_ap = beta[:]
        nc.gpsimd.dma_start(out=bet_sb, in_=bass.AP(
            tensor=b_ap.tensor, offset=b_ap.offset, ap=[[0, 128]] + list(b_ap.ap)))
        eps_sb = const.tile([128, 1], F32)
        nc.vector.memset(eps_sb, LN_EPS)

        # persistent state
        s_run = const.tile([64, B, NS], F32)          # cumsum per b
        hx = const.tile([64, C, B * NS], F32R)        # relu(cumsum) per c
        h_t = const.tile([64, B * NS], F32R)          # GRU hidden
        osum = const.tile([64, B * NS], F32)          # sum over c of h

        # ---------- Phase A: msgT = (adj @ x)^T per (b, c) ----------
        for c in range(C):
            for b in range(B):
                a_in = adj_pool.tile([128, NS // 128, N], F16, tag="a_in")
                nc.gpsimd.dma_start(
                    out=a_in,
                    in_=adj_s[b, c].rearrange("(q p) j -> p q j", p=128))
                ps_acc = psum_a.tile([64, NS], F32, tag="acc")
                for jc in range(JC):
                    ps_tr = psum_t.tile([128, NS // 128, 128], F16, tag="tr")
                    for q in range(NS // 128):
                        nc.tensor.transpose(
                            ps_tr[:, q, :],
                            a_in[:, q, bass.ts(jc, 128)], ident16)
                    adjT = adjt_pool.tile([128, NS // 128, 128], F16, tag="adjT")
                    if jc % 5 < 3:
                        nc.vector.tensor_copy(adjT, ps_tr)
                    else:
                        nc.scalar.copy(adjT, ps_tr)
                    nc.tensor.matmul(
                        ps_acc, x16[:, jc, b, :], adjT,
                        start=(jc == 0), stop=(jc == JC - 1))
                # cumsum + relu
                if c == 0:
                    nc.vector.tensor_copy(s_run[:, b, :], ps_acc)
                else:
                    nc.vector.tensor_add(s_run[:, b, :], s_run[:, b, :], ps_acc)
                nc.vector.tensor_relu(
                    hx[:, c, b * NS:(b + 1) * NS], s_run[:, b, :])

            # ---------- Phase B: GRU step c (both b halves) ----------
            for half in range(2):
                sl = slice(half * NS, (half + 1) * NS)
                hx_c = hx[:, c, sl]
                ps_r = psum.tile([64, NS], F32, tag="ps_r")
                ps_z = psum.tile([64, NS], F32, tag="ps_z")
                ps_n = psum.tile([64, NS], F32, tag="ps_n")
                nc.tensor.matmul(ps_r, wT[:, 0, :], hx_c,
                                 start=True, stop=(c == 0))
                nc.tensor.matmul(ps_z, wT[:, 1, :], hx_c,
                                 start=True, stop=(c == 0))
                nc.tensor.matmul(ps_n, wT[:, 2, :], hx_c, start=True, stop=True)
                if c > 0:
                    nc.tensor.matmul(ps_r, wT[:, 3, :], h_t[:, sl],
                                     start=False, stop=True)
                    nc.tensor.matmul(ps_z, wT[:, 4, :], h_t[:, sl],
                                     start=False, stop=True)
                    ps_hn = psum.tile([64, NS], F32, tag="ps_hn")
                    nc.tensor.matmul(ps_hn, wT[:, 5, :], h_t[:, sl],
                                     start=True, stop=True)
                r_sb = gru.tile([64, NS], F32, tag="r")
                nc.scalar.activation(r_sb, ps_r, AF.Sigmoid, bias=bsum[:, 0:1])
                z_sb = gru.tile([64, NS], F32, tag="z")
                nc.scalar.activation(z_sb, ps_z, AF.Sigmoid, bias=bsum[:, 1:2])
                n_sb = gru.tile([64, NS], F32, tag="n")
                if c > 0:
                    t0 = gru.tile([64, NS], F32, tag="t0")
                    nc.vector.tensor_scalar_add(t0, ps_hn, bhh_sb[:, 2:3])
                    t1 = gru.tile([64, NS], F32, tag="t1")
                    nc.vector.tensor_mul(t1, r_sb, t0)
                    t2 = gru.tile([64, NS], F32, tag="t2")
                    nc.vector.tensor_add(t2, t1, ps_n)
                    nc.scalar.activation(n_sb, t2, AF.Tanh, bias=bih_sb[:, 2:3])
                else:
                    nc.scalar.activation(n_sb, ps_n, AF.Tanh, bias=bih_sb[:, 2:3])
                # h' = n + z*(h - n)   (c=0: h=0 -> h' = n - z*n)
                t3 = gru.tile([64, NS], F32, tag="t3")
                if c > 0:
                    nc.vector.tensor_sub(t3, h_t[:, sl], n_sb)
                else:
                    nc.vector.tensor_scalar_mul(t3, n_sb, -1.0)
                t4 = gru.tile([64, NS], F32, tag="t4")
                nc.vector.tensor_mul(t4, z_sb, t3)
                nc.vector.tensor_add(h_t[:, sl], n_sb, t4)
                if c == 0:
                    nc.vector.tensor_copy(osum[:, sl], h_t[:, sl])
                else:
                    nc.vector.tensor_add(osum[:, sl], osum[:, sl], h_t[:, sl])

        # ---------- Phase C: LayerNorm + output ----------
        oT = const.tile([128, B * NS // 128, H], F32)
        for blk in range(B * NS // 128):
            ps_o = psum_a.tile([128, 64], F32, tag="acc")
            nc.tensor.transpose(ps_o, osum[:, bass.ts(blk, 128)], ident[0:64, 0:64])
            nc.vector.tensor_copy(oT[:, blk, :], ps_o)
        stats = const.tile([128, B * NS // 128, 6], F32)
        mv = const.tile([128, B * NS // 128, 2], F32)
        rstd = const.tile([128, B * NS // 128, 1], F32)
        out_st = const.tile([128, B * NS // 128, H], F32)
        for blk in range(B * NS // 128):
            nc.vector.bn_stats(stats[:, blk, :], oT[:, blk, :])
            nc.vector.bn_aggr(mv[:, blk, :], stats[:, blk, :])
        for blk in range(B * NS // 128):
            nc.scalar.activation(rstd[:, blk, :], mv[:, blk, 1:2],
                                 AF.Sqrt, bias=eps_sb)
        for blk in range(B * NS // 128):
            nc.vector.reciprocal(rstd[:, blk, :], rstd[:, blk, :])
            xm = gru.tile([128, H], F32, tag="xm")
            nc.vector.tensor_scalar_sub(xm, oT[:, blk, :], mv[:, blk, 0:1])
            nc.vector.tensor_scalar_mul(xm, xm, rstd[:, blk, :])
            nc.vector.tensor_mul(xm, xm, gam_sb)
            nc.vector.tensor_add(out_st[:, blk, :], xm, bet_sb)
        for b in range(B):
            nc.sync.dma_start(
                out_s[b].rearrange("(q p) d -> p q d", p=128),
                out_st[:, b * (NS // 128):(b + 1) * (NS // 128), :])

    nc.compile()
    return nc


_NC_CACHE = None


def _get_nc():
    global _NC_CACHE
    if _NC_CACHE is None:
        _NC_CACHE = build()
    return _NC_CACHE


def run(inputs, **spmd_kwargs):
    nc = _get_nc()
    adj = np.ascontiguousarray(inputs["adj"], dtype=np.float32)
    in_maps = []
    for k in range(NCORES):
        m = {
            "adj_s": np.ascontiguousarray(adj[:, :, k * NS:(k + 1) * NS, :]),
            "x": np.ascontiguousarray(inputs["x"], dtype=np.float32),
            "w_ih": np.ascontiguousarray(inputs["w_ih"], dtype=np.float32),
            "w_hh": np.ascontiguousarray(inputs["w_hh"], dtype=np.float32),
            "b_ih": np.ascontiguousarray(inputs["b_ih"], dtype=np.float32),
            "b_hh": np.ascontiguousarray(inputs["b_hh"], dtype=np.float32),
            "gamma": np.ascontiguousarray(inputs["gamma"], dtype=np.float32),
            "beta": np.ascontiguousarray(inputs["beta"], dtype=np.float32),
        }
        in_maps.append(m)
    res = run_bass_kernel_spmd(nc, in_maps, list(range(NCORES)), **spmd_kwargs)
    out = np.concatenate([res.results[k]["out_s"] for k in range(NCORES)], axis=1)
    return out.astype(np.float32), res


def kernel(**inputs):
    out, _ = run(inputs)
    return out


# revision 8
# speedup vs baseline: 466.9588x; 466.9588x over previous
"""Trainium2 Bass kernel for nn_CoreDiffusion (gnn_message_passing).

Sharding: node dim N=4096 split across 8 cores (512 nodes each). Each core:
  msg[b,c] = adj[b,c,rows,:] @ x[b]   (fp16 operands, fp32 PSUM accum)
  hx[c] = relu(cumsum_c msg)          (fp32)
  GRU over c (float32r matmuls), sum over c, LayerNorm (fp32).
No collectives; full output gathered on host.
"""
import numpy as np
from contextlib import ExitStack

import concourse.bass as bass
import concourse.mybir as mybir
import concourse.tile as tile
from concourse import bacc
from concourse.masks import make_identity
from concourse.bass_utils import run_bass_kernel_spmd

F32 = mybir.dt.float32
F32R = mybir.dt.float32r
F16 = mybir.dt.float16
AF = mybir.ActivationFunctionType

B, C, N, D, H = 2, 4, 4096, 64, 64
NCORES = 8
NS = N // NCORES            # 512 nodes per core
JC = N // 128               # 32 contraction chunks
LN_EPS = 1e-5


def build():
    nc = bacc.Bacc("TRN2", target_bir_lowering=False, debug=False,
                   num_devices=NCORES)
    adj_s = nc.declare_dram_parameter("adj_s", [B, C, NS, N], F32, isOutput=False)
    x = nc.declare_dram_parameter("x", [B, N, D], F32, isOutput=False)
    w_ih = nc.declare_dram_parameter("w_ih", [3 * H, D], F32, isOutput=False)
    w_hh = nc.declare_dram_parameter("w_hh", [3 * H, H], F32, isOutput=False)
    b_ih = nc.declare_dram_parameter("b_ih", [3 * H], F32, isOutput=False)
    b_hh = nc.declare_dram_parameter("b_hh", [3 * H], F32, isOutput=False)
    gamma = nc.declare_dram_parameter("gamma", [H], F32, isOutput=False)
    beta = nc.declare_dram_parameter("beta", [H], F32, isOutput=False)
    out_s = nc.declare_dram_parameter("out_s", [B, NS, H], F32, isOutput=True)

    with tile.TileContext(nc) as tc, ExitStack() as ctx:
        const = ctx.enter_context(tc.tile_pool(name="const", bufs=1))
        adj_pool = ctx.enter_context(tc.tile_pool(name="adj", bufs=6))
        adjt_pool = ctx.enter_context(tc.tile_pool(name="adjt", bufs=6))
        gru = ctx.enter_context(tc.tile_pool(name="gru", bufs=2))
        psum = ctx.enter_context(tc.tile_pool(name="psum", bufs=1, space="PSUM"))
        psum_t = ctx.enter_context(tc.tile_pool(name="psum_t", bufs=3, space="PSUM"))
        psum_a = ctx.enter_context(tc.tile_pool(name="psum_a", bufs=1, space="PSUM"))

        # ---------- setup ----------
        ident = const.tile([128, 128], F32)
        make_identity(nc, ident)
        ident16 = const.tile([128, 128], F16)
        nc.vector.tensor_copy(ident16, ident)

        # x -> fp16, layout [j%128, jc, b, d]
        x16 = const.tile([128, JC, B, D], F16)
        for b in range(B):
            nc.gpsimd.dma_start(
                out=x16[:, :, b, :],
                in_=x[b].rearrange("(c p) d -> p c d", p=128))

        # GRU weights: load [192,64] as two partition blocks, PE-transpose gates
        wih_sb = const.tile([128, 2, D], F32)
        nc.sync.dma_start(wih_sb[:, 0, :], w_ih[0:128, :])
        nc.sync.dma_start(wih_sb[0:64, 1, :], w_ih[128:192, :])
        whh_sb = const.tile([128, 2, H], F32)
        nc.sync.dma_start(whh_sb[:, 0, :], w_hh[0:128, :])
        nc.sync.dma_start(whh_sb[0:64, 1, :], w_hh[128:192, :])
        # wT[:, 0:3] = w_ih^T gates r,z,n ; wT[:, 3:6] = w_hh^T
        wT = const.tile([64, 6, 64], F32R)
        for gi, (src, blk, prow) in enumerate([
                (wih_sb, 0, 0), (wih_sb, 0, 64), (wih_sb, 1, 0),
                (whh_sb, 0, 0), (whh_sb, 0, 64), (whh_sb, 1, 0)]):
            ps_w = psum_a.tile([64, 64], F32, tag="acc")
            nc.tensor.transpose(ps_w, src[prow:prow + 64, blk, :],
                                ident[prow:prow + 64, prow:prow + 64])
            nc.vector.tensor_copy(wT[:, gi, :], ps_w)

        # biases as [64, 3] (partition = gate-internal dim)
        bsum = const.tile([64, 3], F32)
        bih_sb = const.tile([64, 3], F32)
        nc.sync.dma_start(bih_sb, b_ih.rearrange("(g p) -> p g", p=64))
        bhh_sb = const.tile([64, 3], F32)
        nc.sync.dma_start(bhh_sb, b_hh.rearrange("(g p) -> p g", p=64))
        nc.vector.tensor_add(bsum, bih_sb, bhh_sb)

        gam_sb = const.tile([128, H], F32)
        g_ap = gamma[:]
        nc.gpsimd.dma_start(out=gam_sb, in_=bass.AP(
            tensor=g_ap.tensor, offset=g_ap.offset, ap=[[0, 128]] + list(g_ap.ap)))
        bet_sb = const.tile([128, H], F32)
        b_ap = beta[:]
        nc.gpsimd.dma_start(out=bet_sb, in_=bass.AP(
            tensor=b_ap.tensor, offset=b_ap.offset, ap=[[0, 128]] + list(b_ap.ap)))
        eps_sb = const.tile([128, 1], F32)
        nc.vector.memset(eps_sb, LN_EPS)

        # persistent state
        s_run = const.tile([64, B, NS], F32)          # cumsum per b
        hx = const.tile([64, C, B * NS], F32R)        # relu(cumsum) per c
        h_t = const.tile([64, B * NS], F32R)          # GRU hidden
        osum = const.tile([64, B * NS], F32)          # sum over c of h

        # ---------- Phase A: msgT = (adj @ x)^T per (b, c) ----------
        NJ = 4                   # j-chunks per DMA
        JW = N // NJ             # 1024 columns per DMA chunk
        for c in range(C):
            for b in range(B):
                src_bc = adj_s[b, c].rearrange("(q p) j -> p q j", p=128)
                ps_acc = psum_a.tile([64, NS], F32, tag="acc")
                for jd in range(NJ):
                    a_in = adj_pool.tile([128, NS // 128, JW], F16, tag="a_in")
                    nc.gpsimd.dma_start(
                        out=a_in,
                        in_=src_bc[:, :, jd * JW:(jd + 1) * JW])
                    for jl in range(JW // 128):
                        jc = jd * (JW // 128) + jl
                        ps_tr = psum_t.tile([128, NS // 128, 128], F16, tag="tr")
                        for q in range(NS // 128):
                            nc.tensor.transpose(
                                ps_tr[:, q, :],
                                a_in[:, q, bass.ts(jl, 128)], ident16)
                        adjT = adjt_pool.tile([128, NS // 128, 128], F16, tag="adjT")
                        if jc % 2 == 0:
                            nc.vector.tensor_copy(adjT, ps_tr)
                        else:
                            nc.scalar.copy(adjT, ps_tr)
                        nc.tensor.matmul(
                            ps_acc, x16[:, jc, b, :], adjT,
                            start=(jc == 0), stop=(jc == JC - 1))
                # cumsum + relu
                if c == 0:
                    nc.vector.tensor_copy(s_run[:, b, :], ps_acc)
                else:
                    nc.vector.tensor_add(s_run[:, b, :], s_run[:, b, :], ps_acc)
                nc.vector.tensor_relu(
                    hx[:, c, b * NS:(b + 1) * NS], s_run[:, b, :])

                # ---------- Phase B: GRU step c, half b ----------
                half = b
                sl = slice(half * NS, (half + 1) * NS)
                hx_c = hx[:, c, sl]
                ps_r = psum.tile([64, NS], F32, tag="ps_r")
                ps_z = psum.tile([64, NS], F32, tag="ps_z")
                ps_n = psum.tile([64, NS], F32, tag="ps_n")
                nc.tensor.matmul(ps_r, wT[:, 0, :], hx_c,
                                 start=True, stop=(c == 0))
                nc.tensor.matmul(ps_z, wT[:, 1, :], hx_c,
                                 start=True, stop=(c == 0))
                nc.tensor.matmul(ps_n, wT[:, 2, :], hx_c, start=True, stop=True)
                if c > 0:
                    nc.tensor.matmul(ps_r, wT[:, 3, :], h_t[:, sl],
                                     start=False, stop=True)
                    nc.tensor.matmul(ps_z, wT[:, 4, :], h_t[:, sl],
                                     start=False, stop=True)
                    ps_hn = psum.tile([64, NS], F32, tag="ps_hn")
                    nc.tensor.matmul(ps_hn, wT[:, 5, :], h_t[:, sl],
                                     start=True, stop=True)
                r_sb = gru.tile([64, NS], F32, tag="r")
                nc.scalar.activation(r_sb, ps_r, AF.Sigmoid, bias=bsum[:, 0:1])
                z_sb = gru.tile([64, NS], F32, tag="z")
                nc.scalar.activation(z_sb, ps_z, AF.Sigmoid, bias=bsum[:, 1:2])
                n_sb = gru.tile([64, NS], F32, tag="n")
                if c > 0:
                    t0 = gru.tile([64, NS], F32, tag="t0")
                    nc.vector.tensor_scalar_add(t0, ps_hn, bhh_sb[:, 2:3])
                    t1 = gru.tile([64, NS], F32, tag="t1")
                    nc.vector.tensor_mul(t1, r_sb, t0)
                    t2 = gru.tile([64, NS], F32, tag="t2")
                    nc.vector.tensor_add(t2, t1, ps_n)
                    nc.scalar.activation(n_sb, t2, AF.Tanh, bias=bih_sb[:, 2:3])
                else:
                    nc.scalar.activation(n_sb, ps_n, AF.Tanh, bias=bih_sb[:, 2:3])
                # h' = n + z*(h - n)   (c=0: h=0 -> h' = n - z*n)
                t3 = gru.tile([64, NS], F32, tag="t3")
                if c > 0:
                    nc.vector.tensor_sub(t3, h_t[:, sl], n_sb)
                else:
                    nc.vector.tensor_scalar_mul(t3, n_sb, -1.0)
                t4 = gru.tile([64, NS], F32, tag="t4")
                nc.vector.tensor_mul(t4, z_sb, t3)
                nc.vector.tensor_add(h_t[:, sl], n_sb, t4)
                if c == 0:
                    nc.vector.tensor_copy(osum[:, sl], h_t[:, sl])
                else:
                    nc.vector.tensor_add(osum[:, sl], osum[:, sl], h_t[:, sl])

        # ---------- Phase C: LayerNorm + output ----------
        oT = const.tile([128, B * NS // 128, H], F32)
        for blk in range(B * NS // 128):
            ps_o = psum_a.tile([128, 64], F32, tag="acc")
            nc.tensor.transpose(ps_o, osum[:, bass.ts(blk, 128)], ident[0:64, 0:64])
            nc.vector.tensor_copy(oT[:, blk, :], ps_o)
        stats = const.tile([128, B * NS // 128, 6], F32)
        mv = const.tile([128, B * NS // 128, 2], F32)
        rstd = const.tile([128, B * NS // 128, 1], F32)
        out_st = const.tile([128, B * NS // 128, H], F32)
        for blk in range(B * NS // 128):
            nc.vector.bn_stats(stats[:, blk, :], oT[:, blk, :])
            nc.vector.bn_aggr(mv[:, blk, :], stats[:, blk, :])
        for blk in range(B * NS // 128):
            nc.scalar.activation(rstd[:, blk, :], mv[:, blk, 1:2],
                                 AF.Sqrt, bias=eps_sb)
        for blk in range(B * NS // 128):
            nc.vector.reciprocal(rstd[:, blk, :], rstd[:, blk, :])
            xm = gru.tile([128, H], F32, tag="xm")
            nc.vector.tensor_scalar_sub(xm, oT[:, blk, :], mv[:, blk, 0:1])
            nc.vector.tensor_scalar_mul(xm, xm, rstd[:, blk, :])
            nc.vector.tensor_mul(xm, xm, gam_sb)
            nc.vector.tensor_add(out_st[:, blk, :], xm, bet_sb)
        for b in range(B):
            nc.sync.dma_start(
                out_s[b].rearrange("(q p) d -> p q d", p=128),
                out_st[:, b * (NS // 128):(b + 1) * (NS // 128), :])

    nc.compile()
    return nc


_NC_CACHE = None


def _get_nc():
    global _NC_CACHE
    if _NC_CACHE is None:
        _NC_CACHE = build()
    return _NC_CACHE


def run(inputs, **spmd_kwargs):
    nc = _get_nc()
    adj = np.ascontiguousarray(inputs["adj"], dtype=np.float32)
    in_maps = []
    for k in range(NCORES):
        m = {
            "adj_s": np.ascontiguousarray(adj[:, :, k * NS:(k + 1) * NS, :]),
            "x": np.ascontiguousarray(inputs["x"], dtype=np.float32),
            "w_ih": np.ascontiguousarray(inputs["w_ih"], dtype=np.float32),
            "w_hh": np.ascontiguousarray(inputs["w_hh"], dtype=np.float32),
            "b_ih": np.ascontiguousarray(inputs["b_ih"], dtype=np.float32),
            "b_hh": np.ascontiguousarray(inputs["b_hh"], dtype=np.float32),
            "gamma": np.ascontiguousarray(inputs["gamma"], dtype=np.float32),
            "beta": np.ascontiguousarray(inputs["beta"], dtype=np.float32),
        }
        in_maps.append(m)
    res = run_bass_kernel_spmd(nc, in_maps, list(range(NCORES)), **spmd_kwargs)
    out = np.concatenate([res.results[k]["out_s"] for k in range(NCORES)], axis=1)
    return out.astype(np.float32), res


def kernel(**inputs):
    out, _ = run(inputs)
    return out
